# revision 60
# baseline (speedup 1.0000x reference)
"""BertAttention (B=32, S=512, H=768, 12 heads) Bass/Tile kernel for 8 TRN2 cores.

Sharding: data-parallel over batch — 4 batches per NeuronCore. kernel() takes
the FULL inputs, slices/preps them on host, runs one SPMD NEFF on cores 0-7,
and reassembles the full (32, 512, 768) output.

All matmuls run in fp8 (e4m3); the deep contractions (Q/K/V projections,
attn*V over keys, O projection) use DoubleRow perf mode — two 128-deep
contraction subtiles per instruction at double rate. The scores matmul
contracts only d=64, so it runs as plain fp8 matmuls with the two heads of a
pair sharing the PE array at partition bases 0/64 (matmul operands may only
start at partition 0/32/64, which rules out a 4x32 d-folded DoubleRow):
  - exp() runs on ACT straight out of the scores psum (scale 1/32 folds the
    1/sqrt(64) and the fp8 q/k x2 scales; bias carries mask - ln4 so exp fits
    fp8 range). Softmax denominators are taken TRANSPOSED (s per q-token on
    partitions) by tiny ones-rhs matmuls off the same exp tiles, so the
    reciprocal is partition-parallel ([64,16] per pair), then broadcast to a
    [64, 1024] tile via a DRAM-bounce DMA; one DVE multiply per pair both
    evacuates the attn*V psum and normalizes, writing the fp8 O-proj operand.
  - softmax bias bv is folded into bo on host (attn weights sum to 1), the
    residual is pre-scaled by 256 = product of all fp8 scale factors (ln is
    scale-invariant), outputs store as bf16 (the LN-normalized values are
    O(1)), and ln_w/ln_b are applied host-side on the output.
"""

import sys

for _p in ("/opt/trn_rl_repo",):
    if _p not in sys.path:
        sys.path.insert(0, _p)

import numpy as np
import ml_dtypes

FP8 = ml_dtypes.float8_e4m3
BF16 = ml_dtypes.bfloat16

N_CORES = 8
B_LOC = 4            # batches per core
S = 512              # sequence length
T = B_LOC * S        # tokens per core
H = 768              # hidden
NH = 12              # heads
D = 64               # head size
KT = 6               # 128-wide hidden tiles
PAIRS = NH // 2      # head pairs (6)
KT4 = S // 128       # 128-wide key-token tiles per batch (4)

_CACHE = {}


def _build():
    import concourse.bacc as bacc
    import concourse.tile as tile
    from concourse import mybir
    import concourse.bass as bass

    f32 = mybir.dt.float32
    bf16 = mybir.dt.bfloat16
    fp8 = mybir.dt.float8e4
    AF = mybir.ActivationFunctionType
    OP = mybir.AluOpType
    PM = mybir.MatmulPerfMode

    nc = bacc.Bacc("TRN2", target_bir_lowering=False, debug=False,
                   enable_asserts=False, num_devices=N_CORES)

    xT_d = nc.dram_tensor("xT", [H, T], fp8, kind="ExternalInput").ap()
    xres_d = nc.dram_tensor("xres", [T, H], bf16, kind="ExternalInput").ap()
    wq_d = nc.dram_tensor("wq", [H, H], fp8, kind="ExternalInput").ap()
    wk_d = nc.dram_tensor("wk", [H, H], fp8, kind="ExternalInput").ap()
    wv_d = nc.dram_tensor("wv", [H, H], fp8, kind="ExternalInput").ap()
    wo_d = nc.dram_tensor("wo", [D, NH, H], fp8, kind="ExternalInput").ap()
    cst_d = nc.dram_tensor("cst", [128, 2 * KT + KT4 * B_LOC], f32,
                           kind="ExternalInput").ap()
    out_d = nc.dram_tensor("out", [T, H], bf16, kind="ExternalOutput").ap()

    xres_t = xres_d.rearrange("(tt p) h -> tt p h", p=128)
    out_t = out_d.rearrange("(tt p) h -> tt p h", p=128)

    with tile.TileContext(nc) as tc:
        with tc.tile_pool(name="persist", bufs=1) as persist, \
             tc.tile_pool(name="qkv", bufs=2) as qkv, \
             tc.tile_pool(name="expp", bufs=5) as expp, \
             tc.tile_pool(name="wtp", bufs=2) as wtp, \
             tc.tile_pool(name="sbcp", bufs=6) as sbcp, \
             tc.tile_pool(name="smalls", bufs=4) as smalls, \
             tc.tile_pool(name="lnp", bufs=3) as lnp, \
             tc.tile_pool(name="yp", bufs=6) as yp, \
             tc.tile_pool(name="drp", bufs=12, space="DRAM") as drp, \
             tc.tile_pool(name="pp", bufs=2, space="PSUM") as pp, \
             tc.tile_pool(name="scp", bufs=2, space="PSUM") as scp, \
             tc.tile_pool(name="wevp", bufs=2, space="PSUM") as wevp:
            # ---- persistent tensors ----
            xT_sb = persist.tile([128, KT, T], fp8)       # [p, kt, tok]
            wq_sb = persist.tile([128, KT, H], fp8)
            wk_sb = persist.tile([128, KT, H], fp8)
            wv_sb = persist.tile([128, KT, H], fp8)
            wo_sb = persist.tile([D, NH, H], fp8)         # [d, head, hid_out]
            cst_sb = persist.tile([128, 2 * KT + KT4 * B_LOC], f32)
            bq2_sb = cst_sb[:, 0:KT]
            bk2_sb = cst_sb[:, KT:2 * KT]
            mask_sb = cst_sb[:, 2 * KT:].rearrange("p (kt b) -> p kt b", b=B_LOC)
            eps_sb = persist.tile([128, 1], f32)
            ones8_sb = persist.tile([128, 2, 1], fp8)     # 0.25: softmax-sum rhs

            xT_t = xT_d.rearrange("(kt p) t -> p kt t", p=128)
            wq_t = wq_d.rearrange("(kt p) j -> p kt j", p=128)
            wk_t = wk_d.rearrange("(kt p) j -> p kt j", p=128)
            wv_t = wv_d.rearrange("(kt p) j -> p kt j", p=128)
            # ordered so batch-0 pair-0 operands land first: tiny tensors,
            # x(b0), then per-jt column chunks of Wq/Wk interleaved with Wv
            nc.sync.dma_start(out=cst_sb, in_=cst_d)
            nc.sync.dma_start(out=xT_sb[:, :, 0:S], in_=xT_t[:, :, 0:S])
            nc.sync.dma_start(out=wq_sb, in_=wq_t)
            nc.sync.dma_start(out=wk_sb, in_=wk_t)
            nc.sync.dma_start(out=wv_sb, in_=wv_t)
            for bb in range(1, B_LOC):
                nc.sync.dma_start(out=xT_sb[:, :, bb * S:(bb + 1) * S],
                                  in_=xT_t[:, :, bb * S:(bb + 1) * S])
            nc.sync.dma_start(out=wo_sb, in_=wo_d)
            nc.vector.memset(eps_sb, 1e-12)
            nc.vector.memset(ones8_sb, 0.25)
            # Pre-load ACT LUT set 6 (natural_log_exp_and_others): holds Exp
            # and Ln, the only ACT functions used, so no table reloads occur.
            _tables = list(__import__("concourse.hw_specs", fromlist=["x"])
                           .get_activation_tables(nc.m.arch))
            _set6 = _tables.index("natural_log_exp_and_others")
            nc.scalar.add_instruction(mybir.InstLoadActFuncSet(
                name=nc.get_next_instruction_name(), ins=[], outs=[],
                act_func_set_id=_set6))

            # ---- per-batch emission helpers (software-pipelined below) ----
            def alloc_qkv():
                qb = qkv.tile([128, PAIRS, S], fp8, tag="qb")  # [p, jt, tok]
                kb = qkv.tile([128, PAIRS, S], fp8, tag="kb")
                vb = qkv.tile([128, KT4, NH, D], fp8, tag="vb")  # [ktok, tl, head, d]
                return qb, kb, vb

            def emit_qk_proj(b, t, w_sb, b_sb, dst, on_act=False):
                """One Q/K psum tile jt=t -> fp8 SBUF with bias. Roughly half
                the evacuations go to ACT (Identity+bias) to balance DVE/ACT."""
                ps = pp.tile([128, S], f32, tag="proj")
                for g in range(3):
                    nc.tensor.matmul(
                        ps, w_sb[:, 2 * g:2 * g + 2, t * 128:(t + 1) * 128],
                        xT_sb[:, 2 * g:2 * g + 2, b * S:(b + 1) * S],
                        start=(g == 0), stop=(g == 2), perf_mode=PM.DoubleRow)
                if on_act:
                    nc.scalar.activation(dst[:, t, :], ps, AF.Identity,
                                         bias=b_sb[:, t:t + 1], scale=2.0 ** -5)
                else:
                    nc.vector.tensor_scalar(out=dst[:, t, :], in0=ps,
                                            scalar1=2.0 ** -5,
                                            scalar2=b_sb[:, t:t + 1],
                                            op0=OP.mult, op1=OP.add)

            def emit_v_group(b, vb, tl, cg):
                n = 512 if cg == 0 else 256
                ps = pp.tile([128, n], f32, tag="proj")
                tt = b * KT4 + tl
                for g in range(3):
                    nc.tensor.matmul(
                        ps, xT_sb[:, 2 * g:2 * g + 2, tt * 128:(tt + 1) * 128],
                        wv_sb[:, 2 * g:2 * g + 2, cg * 512:cg * 512 + n],
                        start=(g == 0), stop=(g == 2), perf_mode=PM.DoubleRow)
                nc.vector.tensor_scalar(out=vb[:, tl, cg * 8:cg * 8 + n // D, :],
                                        in0=ps, scalar1=2.0 ** -6, scalar2=None,
                                        op0=OP.mult)

            V_GROUPS = [(tl, cg) for tl in range(KT4) for cg in range(2)]
            V_SLICE = {0: [0], 1: [1], 2: [2], 3: [3], 4: [4, 5], 5: [6, 7]}

            def emit_proj_slice(b, pr, tiles):
                qb, kb, vb = tiles
                emit_qk_proj(b, pr, wq_sb, bq2_sb, qb)
                emit_qk_proj(b, pr, wk_sb, bk2_sb, kb, on_act=(pr % 2 == 1))
                for g in V_SLICE[pr]:
                    emit_v_group(b, vb, *V_GROUPS[g])

            def emit_o_chunk(ost, qt, split_pp=False):
                """O projection + residual + LN stats for one 128-token tile
                of batch ost['b'] (spread across the NEXT batch's pair loop)."""
                b, wt_sb, xrs = ost["b"], ost["wt"], ost["xrs"]
                y = yp.tile([128, H], f32, tag="y")
                # O psum lives in the wev ring (not pp): pp stays 4-deep per
                # pair (Q/K/V/st) so its 2 slots never gate the ACT evacs.
                # In the tail (split_pp) odd tiles use the pp ring instead so
                # two O projections are in flight.
                if split_pp:
                    op0 = pp.tile([128, 512], f32, tag="proj")
                    op1 = pp.tile([128, 256], f32, tag="proj")
                    pss = [op0, op1]
                else:
                    ops = wevp.tile([128, H], f32, tag="wev", bufs=1)
                    pss = [ops[:, 0:512], ops[:, 512:H]]
                for cg in range(2):
                    n = 512 if cg == 0 else 256
                    for j in range(PAIRS):
                        nc.tensor.matmul(
                            pss[cg],
                            wt_sb[:, 2 * j:2 * j + 2, qt * 128:(qt + 1) * 128],
                            wo_sb[:, 2 * j:2 * j + 2, cg * 512:cg * 512 + n],
                            start=(j == 0), stop=(j == PAIRS - 1),
                            perf_mode=PM.DoubleRow)
                if split_pp:
                    nc.vector.tensor_add(y[:, 0:512], xrs[qt][:, 0:512], pss[0])
                    nc.vector.tensor_add(y[:, 512:H], xrs[qt][:, 512:H], pss[1])
                else:
                    nc.vector.tensor_add(y, xrs[qt], ops)
                ost["ys"].append(y)
                stats = smalls.tile([128, 2, 6], f32, tag="st")
                for g in range(2):
                    nc.vector.bn_stats(stats[:, g, :], y[:, g * 384:(g + 1) * 384])
                nc.vector.bn_aggr(ost["mvb"][:, qt, :], stats)

            def emit_fin(ost, qts, alt_engine=False):
                """LN finalize (rstd + normalize + out DMAs) for tiles qts.
                rstd = exp(-0.5*ln(var+eps)); Ln and Exp share LUT set 6."""
                b, mvb, ys = ost["b"], ost["mvb"], ost["ys"]
                q0, nq = qts[0], len(qts)
                lnv = smalls.tile([128, nq], f32, tag="lnv")
                nc.scalar.activation(lnv, mvb[:, q0:q0 + nq, 1], AF.Ln,
                                     bias=eps_sb, scale=1.0)
                rstd = smalls.tile([128, nq], f32, tag="rstd")
                nc.scalar.activation(rstd, lnv, AF.Exp, bias=0.0, scale=-0.5)
                for i, qt in enumerate(qts):
                    o = lnp.tile([128, H], bf16, tag="o")
                    eng = nc.vector if (alt_engine and qt % 2) else nc.gpsimd
                    eng.tensor_scalar(out=o, in0=ys[qt],
                                      scalar1=mvb[:, qt, 0:1],
                                      scalar2=rstd[:, i:i + 1],
                                      op0=OP.subtract, op1=OP.mult)
                    dge = nc.sync if alt_engine else nc.gpsimd
                    dge.dma_start(out=out_t[b * KT4 + qt], in_=o)

            def emit_scores_exp(b, pr, qb, kb):
                ex = expp.tile([128, KT4, 2 * S], fp8, tag="ex")
                for kt in range(KT4):
                    ps = scp.tile([128, 1024], f32, tag="sc")
                    for hh in range(2):
                        lo, hi = hh * 64, (hh + 1) * 64
                        nc.tensor.matmul(
                            ps[:, hh * 512:(hh + 1) * 512],
                            kb[lo:hi, pr, kt * 128:(kt + 1) * 128],
                            qb[lo:hi, pr, :],
                            start=True, stop=True)
                    nc.scalar.activation(ex[:, kt, :], ps, AF.Exp,
                                         bias=mask_sb[:, kt, b:b + 1],
                                         scale=2.0 ** -5)
                return ex

            def emit_s_chain(ex):
                """Transposed softmax sums sT[q%64, hh*8+qs] = sum_k ex/4 (tiny
                ones-rhs matmuls into the proj psum ring), partition-parallel
                reciprocal, then scatter+broadcast 1/s to a [64, 1024] tile."""
                st = pp.tile([64, 16], f32, tag="proj")
                for hh in range(2):
                    for qs in range(8):
                        c0 = hh * 512 + qs * 64
                        nc.tensor.matmul(
                            st[:, hh * 8 + qs:hh * 8 + qs + 1],
                            ex[:, 0:2, c0:c0 + 64], ones8_sb,
                            start=True, stop=False, perf_mode=PM.DoubleRow)
                        nc.tensor.matmul(
                            st[:, hh * 8 + qs:hh * 8 + qs + 1],
                            ex[:, 2:4, c0:c0 + 64], ones8_sb,
                            start=False, stop=True, perf_mode=PM.DoubleRow)
                str_sb = smalls.tile([64, 16], f32, tag="str", bufs=6)
                nc.vector.reciprocal(str_sb, st)
                # scatter 1/s into final column order dsr[c*64+l], so the
                # broadcast is one clean 2-dim contiguous DMA
                dsr = drp.tile([1, 1024], f32, tag="dsr")
                nc.sync.dma_start(
                    out=bass.AP(tensor=dsr.tensor, offset=dsr.offset,
                                ap=[[1, 64], [64, 16], [1, 1]]),
                    in_=str_sb)
                sbc = sbcp.tile([64, 1024], f32, tag="sbc")
                nc.sync.dma_start(out=sbc, in_=dsr.to_broadcast([64, 1024]))
                return sbc

            def emit_attnv_mult(vb, wt_sb, pr, ex, sbc):
                # both heads into one [64, 1024] psum; single-buffered is fine
                # because attnv(p+1) is emitted a full pair after mult(p)
                wev = wevp.tile([64, 1024], f32, tag="wev", bufs=1)
                for hh in range(2):
                    h = 2 * pr + hh
                    for g in range(2):
                        nc.tensor.matmul(
                            wev[:, hh * 512:(hh + 1) * 512],
                            vb[:, 2 * g:2 * g + 2, h, :],
                            ex[:, 2 * g:2 * g + 2, hh * 512:(hh + 1) * 512],
                            start=(g == 0), stop=(g == 1),
                            perf_mode=PM.DoubleRow)
                nc.vector.tensor_mul(wt_sb[:, 2 * pr:2 * pr + 2, :], wev, sbc)

            # prologue: batch 0 projections
            cur = alloc_qkv()
            for pr in range(PAIRS):
                emit_proj_slice(0, pr, cur)

            pending = None   # o_ln state of the previous batch
            st1 = None       # (vb, wt, pr, ex): awaiting s_chain (1 pair back)
            st2 = None       # (vb, wt, pr, ex, sbc): awaiting attnv (2 back)
            for b in range(B_LOC):
                qb, kb, vb = cur
                nxt = alloc_qkv() if b + 1 < B_LOC else None

                wt_sb = wtp.tile([64, NH, S], fp8, tag="wt")
                xrs = []
                # attn*V + normalize run one pair behind scores/exp (so the
                # 1/s scatter+broadcast DMA chain has a full pair of slack and
                # the DVE stream never head-of-line blocks on it); the
                # PREVIOUS batch's O-projection/LN spreads across pairs 0-4.
                for pr in range(PAIRS):
                    ex = emit_scores_exp(b, pr, qb, kb)
                    if nxt is not None:
                        emit_proj_slice(b + 1, pr, nxt)
                    if st2 is not None:
                        emit_attnv_mult(*st2)
                    if pending is not None:
                        if pr < KT4:
                            emit_o_chunk(pending, pr)
                        elif pr == KT4:
                            emit_fin(pending, range(KT4))
                            pending = None
                    if pr == 3:
                        for qt in range(KT4):
                            xr = lnp.tile([128, H], bf16, tag="xr", bufs=8)
                            nc.gpsimd.dma_start(out=xr, in_=xres_t[b * KT4 + qt])
                            xrs.append(xr)
                    st2 = (vb, wt_sb, pr, ex, emit_s_chain(ex))
                    if b == B_LOC - 1:
                        # last batch: flush same-pair (DVE has slack to absorb
                        # the broadcast wait), so the drain starts earlier
                        emit_attnv_mult(*st2)
                        st2 = None

                mvb = smalls.tile([128, KT4, 2], f32, tag="mvb")
                ost = {"b": b, "wt": wt_sb, "xrs": xrs, "ys": [], "mvb": mvb}
                if b < B_LOC - 1:
                    pending = ost
                else:
                    # tail: per-qt finalize on alternating engines/psum rings
                    for qt in range(KT4):
                        emit_o_chunk(ost, qt, split_pp=(qt % 2 == 1))
                        emit_fin(ost, [qt], alt_engine=True)
                cur = nxt

    nc.compile()
    return nc


def _get_nc():
    if "nc" not in _CACHE:
        _CACHE["nc"] = _build()
    return _CACHE["nc"]


def _prep_in_maps(inputs):
    x = np.asarray(inputs["x"], np.float32)
    mask = np.asarray(inputs["additive_attention_mask"], np.float32)
    Wq = np.asarray(inputs["Wq"], np.float32)
    Wk = np.asarray(inputs["Wk"], np.float32)
    Wv = np.asarray(inputs["Wv"], np.float32)
    Wo = np.asarray(inputs["Wo"], np.float32)
    bq = np.asarray(inputs["bq"], np.float32)
    bk = np.asarray(inputs["bk"], np.float32)
    bv = np.asarray(inputs["bv"], np.float32)
    bo = np.asarray(inputs["bo"], np.float32)

    wq8 = np.ascontiguousarray(Wq.T * 64.0).astype(FP8)
    wk8 = np.ascontiguousarray(Wk.T * 64.0).astype(FP8)
    wv8 = np.ascontiguousarray(Wv.T * 64.0).astype(FP8)
    wo8 = np.ascontiguousarray(
        (Wo.T * 64.0).reshape(NH, D, H).transpose(1, 0, 2)).astype(FP8)
    bq2 = (2.0 * bq).reshape(KT, 128).T
    bk2 = (2.0 * bk).reshape(KT, 128).T
    bo2 = bo + Wo @ bv  # attn weights sum to 1: bv passes through to O-proj

    shared = {"wq": wq8, "wk": wk8, "wv": wv8, "wo": wo8}
    in_maps = []
    for c in range(N_CORES):
        xs = x[c * B_LOC:(c + 1) * B_LOC].reshape(T, H)
        # mask as [128, kt, b]: token k = kt*128+p of batch b, minus ln4
        mc = (mask[c * B_LOC:(c + 1) * B_LOC, 0, 0, :] - np.log(4.0))
        mkb = mc.reshape(B_LOC, KT4, 128).transpose(2, 1, 0).reshape(128, KT4 * B_LOC)
        cst = np.concatenate([bq2, bk2, mkb], axis=1).astype(np.float32)
        in_maps.append({
            "xT": np.ascontiguousarray(xs.T).astype(FP8),
            "xres": (np.ascontiguousarray(xs + bo2[None, :]) * 256.0).astype(BF16),
            "cst": np.ascontiguousarray(cst),
            **shared,
        })
    return in_maps


def run(inputs, trace=False):
    """Returns (full_output, BassKernelResults)."""
    from concourse.bass_utils import run_bass_kernel_spmd

    nc = _get_nc()
    in_maps = _prep_in_maps(inputs)
    res = run_bass_kernel_spmd(nc, in_maps, core_ids=list(range(N_CORES)),
                               trace=trace)
    out = np.concatenate(
        [res.results[c]["out"].astype(np.float32).reshape(B_LOC, S, H)
         for c in range(N_CORES)], axis=0)
    ln_w = np.asarray(inputs["ln_w"], np.float32)
    ln_b = np.asarray(inputs["ln_b"], np.float32)
    out = out * ln_w[None, None, :] + ln_b[None, None, :]
    return np.ascontiguousarray(out.astype(np.float32)), res


def kernel(**inputs) -> np.ndarray:
    out, _ = run(inputs, trace=False)
    return out


# revision 69
# speedup vs baseline: 1.0048x; 1.0048x over previous
"""BertAttention (B=32, S=512, H=768, 12 heads) Bass/Tile kernel for 8 TRN2 cores.

Sharding: data-parallel over batch — 4 batches per NeuronCore. kernel() takes
the FULL inputs, slices/preps them on host, runs one SPMD NEFF on cores 0-7,
and reassembles the full (32, 512, 768) output.

All matmuls run in fp8 (e4m3); the deep contractions (Q/K/V projections,
attn*V over keys, O projection) use DoubleRow perf mode — two 128-deep
contraction subtiles per instruction at double rate. The scores matmul
contracts only d=64, so it runs as plain fp8 matmuls with the two heads of a
pair sharing the PE array at partition bases 0/64 (matmul operands may only
start at partition 0/32/64, which rules out a 4x32 d-folded DoubleRow):
  - exp() runs on ACT straight out of the scores psum (scale 1/32 folds the
    1/sqrt(64) and the fp8 q/k x2 scales; bias carries mask - ln4 so exp fits
    fp8 range). Softmax denominators are taken TRANSPOSED (s per q-token on
    partitions) by tiny ones-rhs matmuls off the same exp tiles, so the
    reciprocal is partition-parallel ([64,16] per pair), then broadcast to a
    [64, 1024] tile via a DRAM-bounce DMA; one DVE multiply per pair both
    evacuates the attn*V psum and normalizes, writing the fp8 O-proj operand.
  - softmax bias bv is folded into bo on host (attn weights sum to 1), the
    residual is pre-scaled by 256 = product of all fp8 scale factors (ln is
    scale-invariant), outputs store as bf16 (the LN-normalized values are
    O(1)), and ln_w/ln_b are applied host-side on the output.
"""

import sys

for _p in ("/opt/trn_rl_repo",):
    if _p not in sys.path:
        sys.path.insert(0, _p)

import numpy as np
import ml_dtypes

FP8 = ml_dtypes.float8_e4m3
BF16 = ml_dtypes.bfloat16

N_CORES = 8
B_LOC = 4            # batches per core
S = 512              # sequence length
T = B_LOC * S        # tokens per core
H = 768              # hidden
NH = 12              # heads
D = 64               # head size
KT = 6               # 128-wide hidden tiles
PAIRS = NH // 2      # head pairs (6)
KT4 = S // 128       # 128-wide key-token tiles per batch (4)

_CACHE = {}


def _build():
    import concourse.bacc as bacc
    import concourse.tile as tile
    from concourse import mybir
    import concourse.bass as bass

    f32 = mybir.dt.float32
    bf16 = mybir.dt.bfloat16
    fp8 = mybir.dt.float8e4
    AF = mybir.ActivationFunctionType
    OP = mybir.AluOpType
    PM = mybir.MatmulPerfMode

    nc = bacc.Bacc("TRN2", target_bir_lowering=False, debug=False,
                   enable_asserts=False, num_devices=N_CORES)

    xT_d = nc.dram_tensor("xT", [H, T], fp8, kind="ExternalInput").ap()
    xres_d = nc.dram_tensor("xres", [T, H], bf16, kind="ExternalInput").ap()
    wq_d = nc.dram_tensor("wq", [H, H], fp8, kind="ExternalInput").ap()
    wk_d = nc.dram_tensor("wk", [H, H], fp8, kind="ExternalInput").ap()
    wv_d = nc.dram_tensor("wv", [H, H], fp8, kind="ExternalInput").ap()
    wo_d = nc.dram_tensor("wo", [D, NH, H], fp8, kind="ExternalInput").ap()
    cst_d = nc.dram_tensor("cst", [128, 2 * KT + KT4 * B_LOC], f32,
                           kind="ExternalInput").ap()
    out_d = nc.dram_tensor("out", [T, H], bf16, kind="ExternalOutput").ap()

    xres_t = xres_d.rearrange("(tt p) h -> tt p h", p=128)
    out_t = out_d.rearrange("(tt p) h -> tt p h", p=128)

    with tile.TileContext(nc) as tc:
        with tc.tile_pool(name="persist", bufs=1) as persist, \
             tc.tile_pool(name="qkv", bufs=2) as qkv, \
             tc.tile_pool(name="expp", bufs=6) as expp, \
             tc.tile_pool(name="wtp", bufs=2) as wtp, \
             tc.tile_pool(name="sbcp", bufs=8) as sbcp, \
             tc.tile_pool(name="smalls", bufs=8) as smalls, \
             tc.tile_pool(name="lnp", bufs=10) as lnp, \
             tc.tile_pool(name="yp", bufs=12) as yp, \
             tc.tile_pool(name="drp", bufs=12, space="DRAM") as drp, \
             tc.tile_pool(name="pp", bufs=2, space="PSUM") as pp, \
             tc.tile_pool(name="scp", bufs=2, space="PSUM") as scp, \
             tc.tile_pool(name="wevp", bufs=2, space="PSUM") as wevp:
            # ---- persistent tensors ----
            xT_sb = persist.tile([128, KT, T], fp8)       # [p, kt, tok]
            wq_sb = persist.tile([128, KT, H], fp8)
            wk_sb = persist.tile([128, KT, H], fp8)
            wv_sb = persist.tile([128, KT, H], fp8)
            wo_sb = persist.tile([D, NH, H], fp8)         # [d, head, hid_out]
            cst_sb = persist.tile([128, 2 * KT + KT4 * B_LOC], f32)
            bq2_sb = cst_sb[:, 0:KT]
            bk2_sb = cst_sb[:, KT:2 * KT]
            mask_sb = cst_sb[:, 2 * KT:].rearrange("p (kt b) -> p kt b", b=B_LOC)
            eps_sb = persist.tile([128, 1], f32)
            ones8_sb = persist.tile([128, 2, 1], fp8)     # 0.25: softmax-sum rhs

            xT_t = xT_d.rearrange("(kt p) t -> p kt t", p=128)
            wq_t = wq_d.rearrange("(kt p) j -> p kt j", p=128)
            wk_t = wk_d.rearrange("(kt p) j -> p kt j", p=128)
            wv_t = wv_d.rearrange("(kt p) j -> p kt j", p=128)
            # ordered so batch-0 pair-0 operands land first: tiny tensors,
            # x(b0), then per-jt column chunks of Wq/Wk interleaved with Wv
            nc.sync.dma_start(out=cst_sb, in_=cst_d)
            nc.sync.dma_start(out=xT_sb[:, :, 0:S], in_=xT_t[:, :, 0:S])
            nc.sync.dma_start(out=wq_sb, in_=wq_t)
            nc.sync.dma_start(out=wk_sb, in_=wk_t)
            nc.sync.dma_start(out=wv_sb, in_=wv_t)
            for bb in range(1, B_LOC):
                nc.sync.dma_start(out=xT_sb[:, :, bb * S:(bb + 1) * S],
                                  in_=xT_t[:, :, bb * S:(bb + 1) * S])
            nc.sync.dma_start(out=wo_sb, in_=wo_d)
            nc.vector.memset(eps_sb, 1e-12)
            nc.vector.memset(ones8_sb, 0.25)
            # Pre-load ACT LUT set 6 (natural_log_exp_and_others): holds Exp
            # and Ln, the only ACT functions used, so no table reloads occur.
            _tables = list(__import__("concourse.hw_specs", fromlist=["x"])
                           .get_activation_tables(nc.m.arch))
            _set6 = _tables.index("natural_log_exp_and_others")
            nc.scalar.add_instruction(mybir.InstLoadActFuncSet(
                name=nc.get_next_instruction_name(), ins=[], outs=[],
                act_func_set_id=_set6))

            # ---- per-batch emission helpers (software-pipelined below) ----
            def alloc_qkv():
                qb = qkv.tile([128, PAIRS, S], fp8, tag="qb")  # [p, jt, tok]
                kb = qkv.tile([128, PAIRS, S], fp8, tag="kb")
                vb = qkv.tile([128, KT4, NH, D], fp8, tag="vb")  # [ktok, tl, head, d]
                return qb, kb, vb

            def emit_qk_proj(b, t, w_sb, b_sb, dst, on_act=False):
                """One Q/K psum tile jt=t -> fp8 SBUF with bias. Roughly half
                the evacuations go to ACT (Identity+bias) to balance DVE/ACT."""
                ps = pp.tile([128, S], f32, tag="proj")
                for g in range(3):
                    nc.tensor.matmul(
                        ps, w_sb[:, 2 * g:2 * g + 2, t * 128:(t + 1) * 128],
                        xT_sb[:, 2 * g:2 * g + 2, b * S:(b + 1) * S],
                        start=(g == 0), stop=(g == 2), perf_mode=PM.DoubleRow)
                if on_act:
                    nc.scalar.activation(dst[:, t, :], ps, AF.Identity,
                                         bias=b_sb[:, t:t + 1], scale=2.0 ** -5)
                else:
                    nc.vector.tensor_scalar(out=dst[:, t, :], in0=ps,
                                            scalar1=2.0 ** -5,
                                            scalar2=b_sb[:, t:t + 1],
                                            op0=OP.mult, op1=OP.add)

            def emit_v_group(b, vb, tl, cg):
                n = 512 if cg == 0 else 256
                ps = pp.tile([128, n], f32, tag="proj")
                tt = b * KT4 + tl
                for g in range(3):
                    nc.tensor.matmul(
                        ps, xT_sb[:, 2 * g:2 * g + 2, tt * 128:(tt + 1) * 128],
                        wv_sb[:, 2 * g:2 * g + 2, cg * 512:cg * 512 + n],
                        start=(g == 0), stop=(g == 2), perf_mode=PM.DoubleRow)
                nc.vector.tensor_scalar(out=vb[:, tl, cg * 8:cg * 8 + n // D, :],
                                        in0=ps, scalar1=2.0 ** -6, scalar2=None,
                                        op0=OP.mult)

            V_GROUPS = [(tl, cg) for tl in range(KT4) for cg in range(2)]
            V_SLICE = {0: [0], 1: [1], 2: [2], 3: [3], 4: [4, 5], 5: [6, 7]}

            def emit_proj_slice(b, pr, tiles):
                qb, kb, vb = tiles
                emit_qk_proj(b, pr, wq_sb, bq2_sb, qb)
                emit_qk_proj(b, pr, wk_sb, bk2_sb, kb, on_act=(pr % 2 == 1))
                for g in V_SLICE[pr]:
                    emit_v_group(b, vb, *V_GROUPS[g])

            def emit_o_chunk(ost, qt, split_pp=False):
                """O projection + residual + LN stats for one 128-token tile
                of batch ost['b'] (spread across the NEXT batch's pair loop)."""
                b, wt_sb, xrs = ost["b"], ost["wt"], ost["xrs"]
                y = yp.tile([128, H], f32, tag="y")
                # O psum lives in the wev ring (not pp): pp stays 4-deep per
                # pair (Q/K/V/st) so its 2 slots never gate the ACT evacs.
                # In the tail (split_pp) odd tiles use the pp ring instead so
                # two O projections are in flight.
                if split_pp:
                    op0 = pp.tile([128, 512], f32, tag="proj")
                    op1 = pp.tile([128, 256], f32, tag="proj")
                    pss = [op0, op1]
                else:
                    ops = wevp.tile([128, H], f32, tag="wev", bufs=1)
                    pss = [ops[:, 0:512], ops[:, 512:H]]
                for cg in range(2):
                    n = 512 if cg == 0 else 256
                    for j in range(PAIRS):
                        nc.tensor.matmul(
                            pss[cg],
                            wt_sb[:, 2 * j:2 * j + 2, qt * 128:(qt + 1) * 128],
                            wo_sb[:, 2 * j:2 * j + 2, cg * 512:cg * 512 + n],
                            start=(j == 0), stop=(j == PAIRS - 1),
                            perf_mode=PM.DoubleRow)
                if split_pp:
                    nc.vector.tensor_add(y[:, 0:512], xrs[qt][:, 0:512], pss[0])
                    nc.vector.tensor_add(y[:, 512:H], xrs[qt][:, 512:H], pss[1])
                else:
                    nc.vector.tensor_add(y, xrs[qt], ops)
                ost["ys"].append(y)
                stats = smalls.tile([128, 2, 6], f32, tag="st")
                for g in range(2):
                    nc.vector.bn_stats(stats[:, g, :], y[:, g * 384:(g + 1) * 384])
                nc.vector.bn_aggr(ost["mvb"][:, qt, :], stats)

            def emit_fin(ost, qts, alt_engine=False):
                """LN finalize (rstd + normalize + out DMAs) for tiles qts.
                rstd = exp(-0.5*ln(var+eps)); Ln and Exp share LUT set 6."""
                b, mvb, ys = ost["b"], ost["mvb"], ost["ys"]
                q0, nq = qts[0], len(qts)
                lnv = smalls.tile([128, nq], f32, tag="lnv")
                nc.scalar.activation(lnv, mvb[:, q0:q0 + nq, 1], AF.Ln,
                                     bias=eps_sb, scale=1.0)
                rstd = smalls.tile([128, nq], f32, tag="rstd")
                nc.scalar.activation(rstd, lnv, AF.Exp, bias=0.0, scale=-0.5)
                for i, qt in enumerate(qts):
                    o = lnp.tile([128, H], bf16, tag="o")
                    eng = nc.vector if (alt_engine and qt % 2) else nc.gpsimd
                    eng.tensor_scalar(out=o, in0=ys[qt],
                                      scalar1=mvb[:, qt, 0:1],
                                      scalar2=rstd[:, i:i + 1],
                                      op0=OP.subtract, op1=OP.mult)
                    dge = nc.sync if alt_engine else nc.gpsimd
                    dge.dma_start(out=out_t[b * KT4 + qt], in_=o)

            def emit_scores_exp(b, pr, qb, kb):
                ex = expp.tile([128, KT4, 2 * S], fp8, tag="ex")
                for kt in range(KT4):
                    ps = scp.tile([128, 1024], f32, tag="sc")
                    for hh in range(2):
                        lo, hi = hh * 64, (hh + 1) * 64
                        nc.tensor.matmul(
                            ps[:, hh * 512:(hh + 1) * 512],
                            kb[lo:hi, pr, kt * 128:(kt + 1) * 128],
                            qb[lo:hi, pr, :],
                            start=True, stop=True)
                    nc.scalar.activation(ex[:, kt, :], ps, AF.Exp,
                                         bias=mask_sb[:, kt, b:b + 1],
                                         scale=2.0 ** -5)
                return ex

            def emit_s_chain(ex):
                """Transposed softmax sums sT[q%64, hh*8+qs] = sum_k ex/4 (tiny
                ones-rhs matmuls into the proj psum ring), partition-parallel
                reciprocal, then scatter+broadcast 1/s to a [64, 1024] tile."""
                st = pp.tile([64, 16], f32, tag="proj")
                for hh in range(2):
                    for qs in range(8):
                        c0 = hh * 512 + qs * 64
                        nc.tensor.matmul(
                            st[:, hh * 8 + qs:hh * 8 + qs + 1],
                            ex[:, 0:2, c0:c0 + 64], ones8_sb,
                            start=True, stop=False, perf_mode=PM.DoubleRow)
                        nc.tensor.matmul(
                            st[:, hh * 8 + qs:hh * 8 + qs + 1],
                            ex[:, 2:4, c0:c0 + 64], ones8_sb,
                            start=False, stop=True, perf_mode=PM.DoubleRow)
                str_sb = smalls.tile([64, 16], f32, tag="str", bufs=8)
                nc.vector.reciprocal(str_sb, st)
                # scatter 1/s into final column order dsr[c*64+l], so the
                # broadcast is one clean 2-dim contiguous DMA
                dsr = drp.tile([1, 1024], f32, tag="dsr")
                nc.sync.dma_start(
                    out=bass.AP(tensor=dsr.tensor, offset=dsr.offset,
                                ap=[[1, 64], [64, 16], [1, 1]]),
                    in_=str_sb)
                sbc = sbcp.tile([64, 1024], f32, tag="sbc")
                nc.sync.dma_start(out=sbc, in_=dsr.to_broadcast([64, 1024]))
                return sbc

            def emit_attnv_mult(vb, wt_sb, pr, ex, sbc):
                # both heads into one [64, 1024] psum; single-buffered is fine
                # because attnv(p+1) is emitted a full pair after mult(p)
                wev = wevp.tile([64, 1024], f32, tag="wev", bufs=1)
                for hh in range(2):
                    h = 2 * pr + hh
                    for g in range(2):
                        nc.tensor.matmul(
                            wev[:, hh * 512:(hh + 1) * 512],
                            vb[:, 2 * g:2 * g + 2, h, :],
                            ex[:, 2 * g:2 * g + 2, hh * 512:(hh + 1) * 512],
                            start=(g == 0), stop=(g == 1),
                            perf_mode=PM.DoubleRow)
                nc.vector.tensor_mul(wt_sb[:, 2 * pr:2 * pr + 2, :], wev, sbc)

            # prologue: batch 0 projections
            cur = alloc_qkv()
            for pr in range(PAIRS):
                emit_proj_slice(0, pr, cur)

            pending = None   # o_ln state of the previous batch
            st1 = None       # (vb, wt, pr, ex): awaiting s_chain (1 pair back)
            st2 = None       # (vb, wt, pr, ex, sbc): awaiting attnv (2 back)
            for b in range(B_LOC):
                qb, kb, vb = cur
                nxt = alloc_qkv() if b + 1 < B_LOC else None

                wt_sb = wtp.tile([64, NH, S], fp8, tag="wt")
                xrs = []
                # attn*V + normalize run one pair behind scores/exp (so the
                # 1/s scatter+broadcast DMA chain has a full pair of slack and
                # the DVE stream never head-of-line blocks on it); the
                # PREVIOUS batch's O-projection/LN spreads across pairs 0-4.
                for pr in range(PAIRS):
                    ex = emit_scores_exp(b, pr, qb, kb)
                    if nxt is not None:
                        emit_proj_slice(b + 1, pr, nxt)
                    if st2 is not None:
                        emit_attnv_mult(*st2)
                    if pending is not None:
                        if pr < KT4:
                            emit_o_chunk(pending, pr)
                        elif pr == KT4:
                            emit_fin(pending, range(KT4))
                            pending = None
                    if pr == 3:
                        for qt in range(KT4):
                            xr = lnp.tile([128, H], bf16, tag="xr", bufs=10)
                            nc.gpsimd.dma_start(out=xr, in_=xres_t[b * KT4 + qt])
                            xrs.append(xr)
                    st2 = (vb, wt_sb, pr, ex, emit_s_chain(ex))
                    if b == B_LOC - 1:
                        # last batch: flush same-pair (DVE has slack to absorb
                        # the broadcast wait), so the drain starts earlier
                        emit_attnv_mult(*st2)
                        st2 = None

                mvb = smalls.tile([128, KT4, 2], f32, tag="mvb")
                ost = {"b": b, "wt": wt_sb, "xrs": xrs, "ys": [], "mvb": mvb}
                if b < B_LOC - 1:
                    pending = ost
                else:
                    # tail: per-qt finalize on alternating engines/psum rings
                    for qt in range(KT4):
                        emit_o_chunk(ost, qt, split_pp=(qt % 2 == 1))
                        emit_fin(ost, [qt], alt_engine=True)
                cur = nxt

    nc.compile()
    return nc


def _get_nc():
    if "nc" not in _CACHE:
        _CACHE["nc"] = _build()
    return _CACHE["nc"]


def _prep_in_maps(inputs):
    x = np.asarray(inputs["x"], np.float32)
    mask = np.asarray(inputs["additive_attention_mask"], np.float32)
    Wq = np.asarray(inputs["Wq"], np.float32)
    Wk = np.asarray(inputs["Wk"], np.float32)
    Wv = np.asarray(inputs["Wv"], np.float32)
    Wo = np.asarray(inputs["Wo"], np.float32)
    bq = np.asarray(inputs["bq"], np.float32)
    bk = np.asarray(inputs["bk"], np.float32)
    bv = np.asarray(inputs["bv"], np.float32)
    bo = np.asarray(inputs["bo"], np.float32)

    wq8 = np.ascontiguousarray(Wq.T * 64.0).astype(FP8)
    wk8 = np.ascontiguousarray(Wk.T * 64.0).astype(FP8)
    wv8 = np.ascontiguousarray(Wv.T * 64.0).astype(FP8)
    wo8 = np.ascontiguousarray(
        (Wo.T * 64.0).reshape(NH, D, H).transpose(1, 0, 2)).astype(FP8)
    bq2 = (2.0 * bq).reshape(KT, 128).T
    bk2 = (2.0 * bk).reshape(KT, 128).T
    bo2 = bo + Wo @ bv  # attn weights sum to 1: bv passes through to O-proj

    shared = {"wq": wq8, "wk": wk8, "wv": wv8, "wo": wo8}
    in_maps = []
    for c in range(N_CORES):
        xs = x[c * B_LOC:(c + 1) * B_LOC].reshape(T, H)
        # mask as [128, kt, b]: token k = kt*128+p of batch b, minus ln4
        mc = (mask[c * B_LOC:(c + 1) * B_LOC, 0, 0, :] - np.log(4.0))
        mkb = mc.reshape(B_LOC, KT4, 128).transpose(2, 1, 0).reshape(128, KT4 * B_LOC)
        cst = np.concatenate([bq2, bk2, mkb], axis=1).astype(np.float32)
        in_maps.append({
            "xT": np.ascontiguousarray(xs.T).astype(FP8),
            "xres": (np.ascontiguousarray(xs + bo2[None, :]) * 256.0).astype(BF16),
            "cst": np.ascontiguousarray(cst),
            **shared,
        })
    return in_maps


def run(inputs, trace=False):
    """Returns (full_output, BassKernelResults)."""
    from concourse.bass_utils import run_bass_kernel_spmd

    nc = _get_nc()
    in_maps = _prep_in_maps(inputs)
    res = run_bass_kernel_spmd(nc, in_maps, core_ids=list(range(N_CORES)),
                               trace=trace)
    out = np.concatenate(
        [res.results[c]["out"].astype(np.float32).reshape(B_LOC, S, H)
         for c in range(N_CORES)], axis=0)
    ln_w = np.asarray(inputs["ln_w"], np.float32)
    ln_b = np.asarray(inputs["ln_b"], np.float32)
    out = out * ln_w[None, None, :] + ln_b[None, None, :]
    return np.ascontiguousarray(out.astype(np.float32)), res


def kernel(**inputs) -> np.ndarray:
    out, _ = run(inputs, trace=False)
    return out


# revision 72
# speedup vs baseline: 1.0094x; 1.0046x over previous
"""BertAttention (B=32, S=512, H=768, 12 heads) Bass/Tile kernel for 8 TRN2 cores.

Sharding: data-parallel over batch — 4 batches per NeuronCore. kernel() takes
the FULL inputs, slices/preps them on host, runs one SPMD NEFF on cores 0-7,
and reassembles the full (32, 512, 768) output.

All matmuls run in fp8 (e4m3); the deep contractions (Q/K/V projections,
attn*V over keys, O projection) use DoubleRow perf mode — two 128-deep
contraction subtiles per instruction at double rate. The scores matmul
contracts only d=64, so it runs as plain fp8 matmuls with the two heads of a
pair sharing the PE array at partition bases 0/64 (matmul operands may only
start at partition 0/32/64, which rules out a 4x32 d-folded DoubleRow):
  - exp() runs on ACT straight out of the scores psum (scale 1/32 folds the
    1/sqrt(64) and the fp8 q/k x2 scales; bias carries mask - ln4 so exp fits
    fp8 range). Softmax denominators are taken TRANSPOSED (s per q-token on
    partitions) by tiny ones-rhs matmuls off the same exp tiles, so the
    reciprocal is partition-parallel ([64,16] per pair), then broadcast to a
    [64, 1024] tile via a DRAM-bounce DMA; one DVE multiply per pair both
    evacuates the attn*V psum and normalizes, writing the fp8 O-proj operand.
  - softmax bias bv is folded into bo on host (attn weights sum to 1), the
    residual is pre-scaled by 256 = product of all fp8 scale factors (ln is
    scale-invariant), outputs store as bf16 (the LN-normalized values are
    O(1)), and ln_w/ln_b are applied host-side on the output.
"""

import sys

for _p in ("/opt/trn_rl_repo",):
    if _p not in sys.path:
        sys.path.insert(0, _p)

import numpy as np
import ml_dtypes

FP8 = ml_dtypes.float8_e4m3
BF16 = ml_dtypes.bfloat16

N_CORES = 8
B_LOC = 4            # batches per core
S = 512              # sequence length
T = B_LOC * S        # tokens per core
H = 768              # hidden
NH = 12              # heads
D = 64               # head size
KT = 6               # 128-wide hidden tiles
PAIRS = NH // 2      # head pairs (6)
KT4 = S // 128       # 128-wide key-token tiles per batch (4)

_CACHE = {}


def _build():
    import concourse.bacc as bacc
    import concourse.tile as tile
    from concourse import mybir
    import concourse.bass as bass

    f32 = mybir.dt.float32
    bf16 = mybir.dt.bfloat16
    fp8 = mybir.dt.float8e4
    AF = mybir.ActivationFunctionType
    OP = mybir.AluOpType
    PM = mybir.MatmulPerfMode

    nc = bacc.Bacc("TRN2", target_bir_lowering=False, debug=False,
                   enable_asserts=False, num_devices=N_CORES)

    xT_d = nc.dram_tensor("xT", [H, T], fp8, kind="ExternalInput").ap()
    xres_d = nc.dram_tensor("xres", [T, H], bf16, kind="ExternalInput").ap()
    wq_d = nc.dram_tensor("wq", [H, H], fp8, kind="ExternalInput").ap()
    wk_d = nc.dram_tensor("wk", [H, H], fp8, kind="ExternalInput").ap()
    wv_d = nc.dram_tensor("wv", [H, H], fp8, kind="ExternalInput").ap()
    wo_d = nc.dram_tensor("wo", [D, NH, H], fp8, kind="ExternalInput").ap()
    cst_d = nc.dram_tensor("cst", [128, 2 * KT + KT4 * B_LOC], f32,
                           kind="ExternalInput").ap()
    out_d = nc.dram_tensor("out", [T, H], bf16, kind="ExternalOutput").ap()

    xres_t = xres_d.rearrange("(tt p) h -> tt p h", p=128)
    out_t = out_d.rearrange("(tt p) h -> tt p h", p=128)

    with tile.TileContext(nc) as tc:
        with tc.tile_pool(name="persist", bufs=1) as persist, \
             tc.tile_pool(name="qkv", bufs=2) as qkv, \
             tc.tile_pool(name="expp", bufs=6) as expp, \
             tc.tile_pool(name="wtp", bufs=2) as wtp, \
             tc.tile_pool(name="sbcp", bufs=8) as sbcp, \
             tc.tile_pool(name="smalls", bufs=8) as smalls, \
             tc.tile_pool(name="lnp", bufs=10) as lnp, \
             tc.tile_pool(name="yp", bufs=12) as yp, \
             tc.tile_pool(name="drp", bufs=12, space="DRAM") as drp, \
             tc.tile_pool(name="pp", bufs=2, space="PSUM") as pp, \
             tc.tile_pool(name="scp", bufs=2, space="PSUM") as scp, \
             tc.tile_pool(name="wevp", bufs=2, space="PSUM") as wevp:
            # ---- persistent tensors ----
            xT_sb = persist.tile([128, KT, T], fp8)       # [p, kt, tok]
            wq_sb = persist.tile([128, KT, H], fp8)
            wk_sb = persist.tile([128, KT, H], fp8)
            wv_sb = persist.tile([128, KT, H], fp8)
            wo_sb = persist.tile([D, NH, H], fp8)         # [d, head, hid_out]
            cst_sb = persist.tile([128, 2 * KT + KT4 * B_LOC], f32)
            bq2_sb = cst_sb[:, 0:KT]
            bk2_sb = cst_sb[:, KT:2 * KT]
            mask_sb = cst_sb[:, 2 * KT:].rearrange("p (kt b) -> p kt b", b=B_LOC)
            eps_sb = persist.tile([128, 1], f32)
            ones8_sb = persist.tile([128, 2, 1], fp8)     # 0.25: softmax-sum rhs

            xT_t = xT_d.rearrange("(kt p) t -> p kt t", p=128)
            wq_t = wq_d.rearrange("(kt p) j -> p kt j", p=128)
            wk_t = wk_d.rearrange("(kt p) j -> p kt j", p=128)
            wv_t = wv_d.rearrange("(kt p) j -> p kt j", p=128)
            # ordered so batch-0 pair-0 operands land first: tiny tensors,
            # x(b0), then per-jt column chunks of Wq/Wk interleaved with Wv
            nc.sync.dma_start(out=cst_sb, in_=cst_d)
            nc.sync.dma_start(out=xT_sb[:, :, 0:S], in_=xT_t[:, :, 0:S])
            nc.sync.dma_start(out=wq_sb, in_=wq_t)
            nc.sync.dma_start(out=wk_sb, in_=wk_t)
            nc.sync.dma_start(out=wv_sb, in_=wv_t)
            for bb in range(1, B_LOC):
                nc.sync.dma_start(out=xT_sb[:, :, bb * S:(bb + 1) * S],
                                  in_=xT_t[:, :, bb * S:(bb + 1) * S])
            nc.sync.dma_start(out=wo_sb, in_=wo_d)
            nc.vector.memset(eps_sb, 1e-12)
            nc.vector.memset(ones8_sb, 0.25)
            # Pre-load ACT LUT set 6 (natural_log_exp_and_others): holds Exp
            # and Ln, the only ACT functions used, so no table reloads occur.
            _tables = list(__import__("concourse.hw_specs", fromlist=["x"])
                           .get_activation_tables(nc.m.arch))
            _set6 = _tables.index("natural_log_exp_and_others")
            nc.scalar.add_instruction(mybir.InstLoadActFuncSet(
                name=nc.get_next_instruction_name(), ins=[], outs=[],
                act_func_set_id=_set6))

            # ---- per-batch emission helpers (software-pipelined below) ----
            def alloc_qkv():
                qb = qkv.tile([128, PAIRS, S], fp8, tag="qb")  # [p, jt, tok]
                kb = qkv.tile([128, PAIRS, S], fp8, tag="kb")
                vb = qkv.tile([128, KT4, NH, D], fp8, tag="vb")  # [ktok, tl, head, d]
                return qb, kb, vb

            def emit_qk_proj(b, t, w_sb, b_sb, dst, on_act=False):
                """One Q/K psum tile jt=t -> fp8 SBUF with bias. Roughly half
                the evacuations go to ACT (Identity+bias) to balance DVE/ACT."""
                ps = pp.tile([128, S], f32, tag="proj")
                for g in range(3):
                    nc.tensor.matmul(
                        ps, w_sb[:, 2 * g:2 * g + 2, t * 128:(t + 1) * 128],
                        xT_sb[:, 2 * g:2 * g + 2, b * S:(b + 1) * S],
                        start=(g == 0), stop=(g == 2), perf_mode=PM.DoubleRow)
                if on_act:
                    nc.scalar.activation(dst[:, t, :], ps, AF.Identity,
                                         bias=b_sb[:, t:t + 1], scale=2.0 ** -5)
                else:
                    nc.vector.tensor_scalar(out=dst[:, t, :], in0=ps,
                                            scalar1=2.0 ** -5,
                                            scalar2=b_sb[:, t:t + 1],
                                            op0=OP.mult, op1=OP.add)

            def emit_v_group(b, vb, tl, cg):
                n = 512 if cg == 0 else 256
                ps = pp.tile([128, n], f32, tag="proj")
                tt = b * KT4 + tl
                for g in range(3):
                    nc.tensor.matmul(
                        ps, xT_sb[:, 2 * g:2 * g + 2, tt * 128:(tt + 1) * 128],
                        wv_sb[:, 2 * g:2 * g + 2, cg * 512:cg * 512 + n],
                        start=(g == 0), stop=(g == 2), perf_mode=PM.DoubleRow)
                nc.vector.tensor_scalar(out=vb[:, tl, cg * 8:cg * 8 + n // D, :],
                                        in0=ps, scalar1=2.0 ** -6, scalar2=None,
                                        op0=OP.mult)

            V_GROUPS = [(tl, cg) for tl in range(KT4) for cg in range(2)]
            V_SLICE = {0: [0], 1: [1], 2: [2], 3: [3], 4: [4, 5], 5: [6, 7]}

            def emit_proj_slice(b, pr, tiles):
                qb, kb, vb = tiles
                emit_qk_proj(b, pr, wq_sb, bq2_sb, qb)
                emit_qk_proj(b, pr, wk_sb, bk2_sb, kb, on_act=(pr % 2 == 1))
                for g in V_SLICE[pr]:
                    emit_v_group(b, vb, *V_GROUPS[g])

            def emit_o_chunk(ost, qt, split_pp=False):
                """O projection + residual + LN stats for one 128-token tile
                of batch ost['b'] (spread across the NEXT batch's pair loop)."""
                b, wt_sb, xrs = ost["b"], ost["wt"], ost["xrs"]
                y = yp.tile([128, H], f32, tag="y")
                # O psum lives in the wev ring (not pp): pp stays 4-deep per
                # pair (Q/K/V/st) so its 2 slots never gate the ACT evacs.
                # In the tail (split_pp) odd tiles use the pp ring instead so
                # two O projections are in flight.
                if split_pp:
                    op0 = pp.tile([128, 512], f32, tag="proj")
                    op1 = pp.tile([128, 256], f32, tag="proj")
                    pss = [op0, op1]
                else:
                    ops = wevp.tile([128, H], f32, tag="wev", bufs=1)
                    pss = [ops[:, 0:512], ops[:, 512:H]]
                for cg in range(2):
                    n = 512 if cg == 0 else 256
                    for j in range(PAIRS):
                        nc.tensor.matmul(
                            pss[cg],
                            wt_sb[:, 2 * j:2 * j + 2, qt * 128:(qt + 1) * 128],
                            wo_sb[:, 2 * j:2 * j + 2, cg * 512:cg * 512 + n],
                            start=(j == 0), stop=(j == PAIRS - 1),
                            perf_mode=PM.DoubleRow)
                if split_pp:
                    nc.vector.tensor_add(y[:, 0:512], xrs[qt][:, 0:512], pss[0])
                    nc.vector.tensor_add(y[:, 512:H], xrs[qt][:, 512:H], pss[1])
                else:
                    nc.vector.tensor_add(y, xrs[qt], ops)
                ost["ys"].append(y)
                stats = smalls.tile([128, 2, 6], f32, tag="st")
                for g in range(2):
                    nc.vector.bn_stats(stats[:, g, :], y[:, g * 384:(g + 1) * 384])
                nc.vector.bn_aggr(ost["mvb"][:, qt, :], stats)

            def emit_fin(ost, qts, alt_engine=False):
                """LN finalize (rstd + normalize + out DMAs) for tiles qts.
                rstd = exp(-0.5*ln(var+eps)); Ln and Exp share LUT set 6."""
                b, mvb, ys = ost["b"], ost["mvb"], ost["ys"]
                q0, nq = qts[0], len(qts)
                lnv = smalls.tile([128, nq], f32, tag="lnv")
                nc.scalar.activation(lnv, mvb[:, q0:q0 + nq, 1], AF.Ln,
                                     bias=eps_sb, scale=1.0)
                rstd = smalls.tile([128, nq], f32, tag="rstd")
                nc.scalar.activation(rstd, lnv, AF.Exp, bias=0.0, scale=-0.5)
                for i, qt in enumerate(qts):
                    o = lnp.tile([128, H], bf16, tag="o")
                    eng = nc.vector if (alt_engine and qt % 2) else nc.gpsimd
                    eng.tensor_scalar(out=o, in0=ys[qt],
                                      scalar1=mvb[:, qt, 0:1],
                                      scalar2=rstd[:, i:i + 1],
                                      op0=OP.subtract, op1=OP.mult)
                    dge = nc.sync if alt_engine else nc.gpsimd
                    dge.dma_start(out=out_t[b * KT4 + qt], in_=o)

            def emit_scores_exp(b, pr, qb, kb):
                ex = expp.tile([128, KT4, 2 * S], fp8, tag="ex")
                for kt in range(KT4):
                    ps = scp.tile([128, 1024], f32, tag="sc")
                    for hh in range(2):
                        lo, hi = hh * 64, (hh + 1) * 64
                        nc.tensor.matmul(
                            ps[:, hh * 512:(hh + 1) * 512],
                            kb[lo:hi, pr, kt * 128:(kt + 1) * 128],
                            qb[lo:hi, pr, :],
                            start=True, stop=True)
                    nc.scalar.activation(ex[:, kt, :], ps, AF.Exp,
                                         bias=mask_sb[:, kt, b:b + 1],
                                         scale=2.0 ** -5)
                return ex

            def emit_s_chain(ex):
                """Transposed softmax sums sT[q%64, hh*8+qs] = sum_k ex/4 (tiny
                ones-rhs matmuls into the proj psum ring), partition-parallel
                reciprocal, then scatter+broadcast 1/s to a [64, 1024] tile."""
                st = pp.tile([64, 16], f32, tag="proj")
                for hh in range(2):
                    for qs in range(8):
                        c0 = hh * 512 + qs * 64
                        nc.tensor.matmul(
                            st[:, hh * 8 + qs:hh * 8 + qs + 1],
                            ex[:, 0:2, c0:c0 + 64], ones8_sb,
                            start=True, stop=False, perf_mode=PM.DoubleRow)
                        nc.tensor.matmul(
                            st[:, hh * 8 + qs:hh * 8 + qs + 1],
                            ex[:, 2:4, c0:c0 + 64], ones8_sb,
                            start=False, stop=True, perf_mode=PM.DoubleRow)
                str_sb = smalls.tile([64, 16], f32, tag="str", bufs=8)
                nc.vector.reciprocal(str_sb, st)
                # scatter 1/s into final column order dsr[c*64+l], so the
                # broadcast is one clean 2-dim contiguous DMA
                dsr = drp.tile([1, 1024], f32, tag="dsr")
                nc.sync.dma_start(
                    out=bass.AP(tensor=dsr.tensor, offset=dsr.offset,
                                ap=[[1, 64], [64, 16], [1, 1]]),
                    in_=str_sb)
                sbc = sbcp.tile([64, 1024], f32, tag="sbc")
                nc.sync.dma_start(out=sbc, in_=dsr.to_broadcast([64, 1024]))
                return sbc

            def emit_attnv_mult(vb, wt_sb, pr, ex, sbc):
                # both heads into one [64, 1024] psum; single-buffered is fine
                # because attnv(p+1) is emitted a full pair after mult(p)
                wev = wevp.tile([64, 1024], f32, tag="wev", bufs=1)
                for hh in range(2):
                    h = 2 * pr + hh
                    for g in range(2):
                        nc.tensor.matmul(
                            wev[:, hh * 512:(hh + 1) * 512],
                            vb[:, 2 * g:2 * g + 2, h, :],
                            ex[:, 2 * g:2 * g + 2, hh * 512:(hh + 1) * 512],
                            start=(g == 0), stop=(g == 1),
                            perf_mode=PM.DoubleRow)
                nc.vector.tensor_mul(wt_sb[:, 2 * pr:2 * pr + 2, :], wev, sbc)

            # prologue: batch 0 projections
            cur = alloc_qkv()
            for pr in range(PAIRS):
                emit_proj_slice(0, pr, cur)

            pending = None   # o_ln state of the previous batch
            st1 = None       # (vb, wt, pr, ex): awaiting s_chain (1 pair back)
            st2 = None       # (vb, wt, pr, ex, sbc): awaiting attnv (2 back)
            for b in range(B_LOC):
                qb, kb, vb = cur
                nxt = alloc_qkv() if b + 1 < B_LOC else None

                wt_sb = wtp.tile([64, NH, S], fp8, tag="wt")
                xrs = []
                # attn*V + normalize run one pair behind scores/exp (so the
                # 1/s scatter+broadcast DMA chain has a full pair of slack and
                # the DVE stream never head-of-line blocks on it); the
                # PREVIOUS batch's O-projection/LN spreads across pairs 0-4.
                for pr in range(PAIRS):
                    ex = emit_scores_exp(b, pr, qb, kb)
                    if nxt is not None:
                        emit_proj_slice(b + 1, pr, nxt)
                    if st2 is not None:
                        emit_attnv_mult(*st2)
                    if pending is not None:
                        if pr < KT4:
                            emit_o_chunk(pending, pr)
                        elif pr == KT4:
                            emit_fin(pending, range(KT4))
                            pending = None
                    if pr == 3:
                        # batch 0's loads go behind the prologue on sync, and
                        # the 4-deep ring makes later batches' dispatches wait
                        # for consumption — otherwise the Pool DGE issues all
                        # 16 at t=0 and their transfers cut into the DMA FIFO
                        # ahead of the weight loads
                        dge = nc.sync if b == 0 else nc.gpsimd
                        for qt in range(KT4):
                            xr = lnp.tile([128, H], bf16, tag="xr", bufs=4)
                            dge.dma_start(out=xr, in_=xres_t[b * KT4 + qt])
                            xrs.append(xr)
                    st2 = (vb, wt_sb, pr, ex, emit_s_chain(ex))
                    if b == B_LOC - 1:
                        # last batch: flush same-pair (DVE has slack to absorb
                        # the broadcast wait), so the drain starts earlier
                        emit_attnv_mult(*st2)
                        st2 = None

                mvb = smalls.tile([128, KT4, 2], f32, tag="mvb")
                ost = {"b": b, "wt": wt_sb, "xrs": xrs, "ys": [], "mvb": mvb}
                if b < B_LOC - 1:
                    pending = ost
                else:
                    # tail: per-qt finalize on alternating engines/psum rings
                    for qt in range(KT4):
                        emit_o_chunk(ost, qt, split_pp=(qt % 2 == 1))
                        emit_fin(ost, [qt], alt_engine=True)
                cur = nxt

    nc.compile()
    return nc


def _get_nc():
    if "nc" not in _CACHE:
        _CACHE["nc"] = _build()
    return _CACHE["nc"]


def _prep_in_maps(inputs):
    x = np.asarray(inputs["x"], np.float32)
    mask = np.asarray(inputs["additive_attention_mask"], np.float32)
    Wq = np.asarray(inputs["Wq"], np.float32)
    Wk = np.asarray(inputs["Wk"], np.float32)
    Wv = np.asarray(inputs["Wv"], np.float32)
    Wo = np.asarray(inputs["Wo"], np.float32)
    bq = np.asarray(inputs["bq"], np.float32)
    bk = np.asarray(inputs["bk"], np.float32)
    bv = np.asarray(inputs["bv"], np.float32)
    bo = np.asarray(inputs["bo"], np.float32)

    wq8 = np.ascontiguousarray(Wq.T * 64.0).astype(FP8)
    wk8 = np.ascontiguousarray(Wk.T * 64.0).astype(FP8)
    wv8 = np.ascontiguousarray(Wv.T * 64.0).astype(FP8)
    wo8 = np.ascontiguousarray(
        (Wo.T * 64.0).reshape(NH, D, H).transpose(1, 0, 2)).astype(FP8)
    bq2 = (2.0 * bq).reshape(KT, 128).T
    bk2 = (2.0 * bk).reshape(KT, 128).T
    bo2 = bo + Wo @ bv  # attn weights sum to 1: bv passes through to O-proj

    shared = {"wq": wq8, "wk": wk8, "wv": wv8, "wo": wo8}
    in_maps = []
    for c in range(N_CORES):
        xs = x[c * B_LOC:(c + 1) * B_LOC].reshape(T, H)
        # mask as [128, kt, b]: token k = kt*128+p of batch b, minus ln4
        mc = (mask[c * B_LOC:(c + 1) * B_LOC, 0, 0, :] - np.log(4.0))
        mkb = mc.reshape(B_LOC, KT4, 128).transpose(2, 1, 0).reshape(128, KT4 * B_LOC)
        cst = np.concatenate([bq2, bk2, mkb], axis=1).astype(np.float32)
        in_maps.append({
            "xT": np.ascontiguousarray(xs.T).astype(FP8),
            "xres": (np.ascontiguousarray(xs + bo2[None, :]) * 256.0).astype(BF16),
            "cst": np.ascontiguousarray(cst),
            **shared,
        })
    return in_maps


def run(inputs, trace=False):
    """Returns (full_output, BassKernelResults)."""
    from concourse.bass_utils import run_bass_kernel_spmd

    nc = _get_nc()
    in_maps = _prep_in_maps(inputs)
    res = run_bass_kernel_spmd(nc, in_maps, core_ids=list(range(N_CORES)),
                               trace=trace)
    out = np.concatenate(
        [res.results[c]["out"].astype(np.float32).reshape(B_LOC, S, H)
         for c in range(N_CORES)], axis=0)
    ln_w = np.asarray(inputs["ln_w"], np.float32)
    ln_b = np.asarray(inputs["ln_b"], np.float32)
    out = out * ln_w[None, None, :] + ln_b[None, None, :]
    return np.ascontiguousarray(out.astype(np.float32)), res


def kernel(**inputs) -> np.ndarray:
    out, _ = run(inputs, trace=False)
    return out


# revision 73
# speedup vs baseline: 1.0139x; 1.0045x over previous
"""BertAttention (B=32, S=512, H=768, 12 heads) Bass/Tile kernel for 8 TRN2 cores.

Sharding: data-parallel over batch — 4 batches per NeuronCore. kernel() takes
the FULL inputs, slices/preps them on host, runs one SPMD NEFF on cores 0-7,
and reassembles the full (32, 512, 768) output.

All matmuls run in fp8 (e4m3); the deep contractions (Q/K/V projections,
attn*V over keys, O projection) use DoubleRow perf mode — two 128-deep
contraction subtiles per instruction at double rate. The scores matmul
contracts only d=64, so it runs as plain fp8 matmuls with the two heads of a
pair sharing the PE array at partition bases 0/64 (matmul operands may only
start at partition 0/32/64, which rules out a 4x32 d-folded DoubleRow):
  - exp() runs on ACT straight out of the scores psum (scale 1/32 folds the
    1/sqrt(64) and the fp8 q/k x2 scales; bias carries mask - ln4 so exp fits
    fp8 range). Softmax denominators are taken TRANSPOSED (s per q-token on
    partitions) by tiny ones-rhs matmuls off the same exp tiles, so the
    reciprocal is partition-parallel ([64,16] per pair), then broadcast to a
    [64, 1024] tile via a DRAM-bounce DMA; one DVE multiply per pair both
    evacuates the attn*V psum and normalizes, writing the fp8 O-proj operand.
  - softmax bias bv is folded into bo on host (attn weights sum to 1), the
    residual is pre-scaled by 256 = product of all fp8 scale factors (ln is
    scale-invariant), outputs store as bf16 (the LN-normalized values are
    O(1)), and ln_w/ln_b are applied host-side on the output.
"""

import sys

for _p in ("/opt/trn_rl_repo",):
    if _p not in sys.path:
        sys.path.insert(0, _p)

import numpy as np
import ml_dtypes

FP8 = ml_dtypes.float8_e4m3
BF16 = ml_dtypes.bfloat16

N_CORES = 8
B_LOC = 4            # batches per core
S = 512              # sequence length
T = B_LOC * S        # tokens per core
H = 768              # hidden
NH = 12              # heads
D = 64               # head size
KT = 6               # 128-wide hidden tiles
PAIRS = NH // 2      # head pairs (6)
KT4 = S // 128       # 128-wide key-token tiles per batch (4)

_CACHE = {}


def _build():
    import concourse.bacc as bacc
    import concourse.tile as tile
    from concourse import mybir
    import concourse.bass as bass

    f32 = mybir.dt.float32
    bf16 = mybir.dt.bfloat16
    fp8 = mybir.dt.float8e4
    AF = mybir.ActivationFunctionType
    OP = mybir.AluOpType
    PM = mybir.MatmulPerfMode

    nc = bacc.Bacc("TRN2", target_bir_lowering=False, debug=False,
                   enable_asserts=False, num_devices=N_CORES)

    xT_d = nc.dram_tensor("xT", [H, T], fp8, kind="ExternalInput").ap()
    xres_d = nc.dram_tensor("xres", [T, H], bf16, kind="ExternalInput").ap()
    wq_d = nc.dram_tensor("wq", [H, H], fp8, kind="ExternalInput").ap()
    wk_d = nc.dram_tensor("wk", [H, H], fp8, kind="ExternalInput").ap()
    wv_d = nc.dram_tensor("wv", [H, H], fp8, kind="ExternalInput").ap()
    wo_d = nc.dram_tensor("wo", [D, NH, H], fp8, kind="ExternalInput").ap()
    cst_d = nc.dram_tensor("cst", [128, 2 * KT + KT4 * B_LOC], f32,
                           kind="ExternalInput").ap()
    out_d = nc.dram_tensor("out", [T, H], bf16, kind="ExternalOutput").ap()

    xres_t = xres_d.rearrange("(tt p) h -> tt p h", p=128)
    out_t = out_d.rearrange("(tt p) h -> tt p h", p=128)

    with tile.TileContext(nc) as tc:
        with tc.tile_pool(name="persist", bufs=1) as persist, \
             tc.tile_pool(name="qkv", bufs=2) as qkv, \
             tc.tile_pool(name="expp", bufs=6) as expp, \
             tc.tile_pool(name="wtp", bufs=2) as wtp, \
             tc.tile_pool(name="sbcp", bufs=8) as sbcp, \
             tc.tile_pool(name="smalls", bufs=8) as smalls, \
             tc.tile_pool(name="lnp", bufs=10) as lnp, \
             tc.tile_pool(name="yp", bufs=12) as yp, \
             tc.tile_pool(name="drp", bufs=12, space="DRAM") as drp, \
             tc.tile_pool(name="pp", bufs=2, space="PSUM") as pp, \
             tc.tile_pool(name="scp", bufs=2, space="PSUM") as scp, \
             tc.tile_pool(name="wevp", bufs=2, space="PSUM") as wevp:
            # ---- persistent tensors ----
            xT_sb = persist.tile([128, KT, T], fp8)       # [p, kt, tok]
            wq_sb = persist.tile([128, KT, H], fp8)
            wk_sb = persist.tile([128, KT, H], fp8)
            wv_sb = persist.tile([128, KT, H], fp8)
            wo_sb = persist.tile([D, NH, H], fp8)         # [d, head, hid_out]
            cst_sb = persist.tile([128, 2 * KT + KT4 * B_LOC], f32)
            bq2_sb = cst_sb[:, 0:KT]
            bk2_sb = cst_sb[:, KT:2 * KT]
            mask_sb = cst_sb[:, 2 * KT:].rearrange("p (kt b) -> p kt b", b=B_LOC)
            eps_sb = persist.tile([128, 1], f32)
            ones8_sb = persist.tile([128, 2, 1], fp8)     # 0.25: softmax-sum rhs

            xT_t = xT_d.rearrange("(kt p) t -> p kt t", p=128)
            wq_t = wq_d.rearrange("(kt p) j -> p kt j", p=128)
            wk_t = wk_d.rearrange("(kt p) j -> p kt j", p=128)
            wv_t = wv_d.rearrange("(kt p) j -> p kt j", p=128)
            # ordered so batch-0 pair-0 operands land first: tiny tensors,
            # x(b0), then per-jt column chunks of Wq/Wk interleaved with Wv
            nc.sync.dma_start(out=xT_sb[:, :, 0:S], in_=xT_t[:, :, 0:S])
            nc.sync.dma_start(out=wq_sb, in_=wq_t)
            nc.sync.dma_start(out=wk_sb, in_=wk_t)
            nc.sync.dma_start(out=cst_sb, in_=cst_d)
            nc.sync.dma_start(out=wv_sb, in_=wv_t)
            for bb in range(1, B_LOC):
                nc.sync.dma_start(out=xT_sb[:, :, bb * S:(bb + 1) * S],
                                  in_=xT_t[:, :, bb * S:(bb + 1) * S])
            nc.sync.dma_start(out=wo_sb, in_=wo_d)
            nc.vector.memset(eps_sb, 1e-12)
            nc.vector.memset(ones8_sb, 0.25)
            # Pre-load ACT LUT set 6 (natural_log_exp_and_others): holds Exp
            # and Ln, the only ACT functions used, so no table reloads occur.
            _tables = list(__import__("concourse.hw_specs", fromlist=["x"])
                           .get_activation_tables(nc.m.arch))
            _set6 = _tables.index("natural_log_exp_and_others")
            nc.scalar.add_instruction(mybir.InstLoadActFuncSet(
                name=nc.get_next_instruction_name(), ins=[], outs=[],
                act_func_set_id=_set6))

            # ---- per-batch emission helpers (software-pipelined below) ----
            def alloc_qkv():
                qb = qkv.tile([128, PAIRS, S], fp8, tag="qb")  # [p, jt, tok]
                kb = qkv.tile([128, PAIRS, S], fp8, tag="kb")
                vb = qkv.tile([128, KT4, NH, D], fp8, tag="vb")  # [ktok, tl, head, d]
                return qb, kb, vb

            def emit_qk_proj(b, t, w_sb, b_sb, dst, on_act=False):
                """One Q/K psum tile jt=t -> fp8 SBUF with bias. Roughly half
                the evacuations go to ACT (Identity+bias) to balance DVE/ACT."""
                ps = pp.tile([128, S], f32, tag="proj")
                for g in range(3):
                    nc.tensor.matmul(
                        ps, w_sb[:, 2 * g:2 * g + 2, t * 128:(t + 1) * 128],
                        xT_sb[:, 2 * g:2 * g + 2, b * S:(b + 1) * S],
                        start=(g == 0), stop=(g == 2), perf_mode=PM.DoubleRow)
                if on_act:
                    nc.scalar.activation(dst[:, t, :], ps, AF.Identity,
                                         bias=b_sb[:, t:t + 1], scale=2.0 ** -5)
                else:
                    nc.vector.tensor_scalar(out=dst[:, t, :], in0=ps,
                                            scalar1=2.0 ** -5,
                                            scalar2=b_sb[:, t:t + 1],
                                            op0=OP.mult, op1=OP.add)

            def emit_v_group(b, vb, tl, cg):
                n = 512 if cg == 0 else 256
                ps = pp.tile([128, n], f32, tag="proj")
                tt = b * KT4 + tl
                for g in range(3):
                    nc.tensor.matmul(
                        ps, xT_sb[:, 2 * g:2 * g + 2, tt * 128:(tt + 1) * 128],
                        wv_sb[:, 2 * g:2 * g + 2, cg * 512:cg * 512 + n],
                        start=(g == 0), stop=(g == 2), perf_mode=PM.DoubleRow)
                nc.vector.tensor_scalar(out=vb[:, tl, cg * 8:cg * 8 + n // D, :],
                                        in0=ps, scalar1=2.0 ** -6, scalar2=None,
                                        op0=OP.mult)

            V_GROUPS = [(tl, cg) for tl in range(KT4) for cg in range(2)]
            V_SLICE = {0: [0], 1: [1], 2: [2], 3: [3], 4: [4, 5], 5: [6, 7]}

            def emit_proj_slice(b, pr, tiles):
                qb, kb, vb = tiles
                emit_qk_proj(b, pr, wq_sb, bq2_sb, qb)
                emit_qk_proj(b, pr, wk_sb, bk2_sb, kb, on_act=(pr % 2 == 1))
                for g in V_SLICE[pr]:
                    emit_v_group(b, vb, *V_GROUPS[g])

            def emit_o_chunk(ost, qt, split_pp=False):
                """O projection + residual + LN stats for one 128-token tile
                of batch ost['b'] (spread across the NEXT batch's pair loop)."""
                b, wt_sb, xrs = ost["b"], ost["wt"], ost["xrs"]
                y = yp.tile([128, H], f32, tag="y")
                # O psum lives in the wev ring (not pp): pp stays 4-deep per
                # pair (Q/K/V/st) so its 2 slots never gate the ACT evacs.
                # In the tail (split_pp) odd tiles use the pp ring instead so
                # two O projections are in flight.
                if split_pp:
                    op0 = pp.tile([128, 512], f32, tag="proj")
                    op1 = pp.tile([128, 256], f32, tag="proj")
                    pss = [op0, op1]
                else:
                    ops = wevp.tile([128, H], f32, tag="wev", bufs=1)
                    pss = [ops[:, 0:512], ops[:, 512:H]]
                for cg in range(2):
                    n = 512 if cg == 0 else 256
                    for j in range(PAIRS):
                        nc.tensor.matmul(
                            pss[cg],
                            wt_sb[:, 2 * j:2 * j + 2, qt * 128:(qt + 1) * 128],
                            wo_sb[:, 2 * j:2 * j + 2, cg * 512:cg * 512 + n],
                            start=(j == 0), stop=(j == PAIRS - 1),
                            perf_mode=PM.DoubleRow)
                if split_pp:
                    nc.vector.tensor_add(y[:, 0:512], xrs[qt][:, 0:512], pss[0])
                    nc.vector.tensor_add(y[:, 512:H], xrs[qt][:, 512:H], pss[1])
                else:
                    nc.vector.tensor_add(y, xrs[qt], ops)
                ost["ys"].append(y)
                stats = smalls.tile([128, 2, 6], f32, tag="st")
                for g in range(2):
                    nc.vector.bn_stats(stats[:, g, :], y[:, g * 384:(g + 1) * 384])
                nc.vector.bn_aggr(ost["mvb"][:, qt, :], stats)

            def emit_fin(ost, qts, alt_engine=False):
                """LN finalize (rstd + normalize + out DMAs) for tiles qts.
                rstd = exp(-0.5*ln(var+eps)); Ln and Exp share LUT set 6."""
                b, mvb, ys = ost["b"], ost["mvb"], ost["ys"]
                q0, nq = qts[0], len(qts)
                lnv = smalls.tile([128, nq], f32, tag="lnv")
                nc.scalar.activation(lnv, mvb[:, q0:q0 + nq, 1], AF.Ln,
                                     bias=eps_sb, scale=1.0)
                rstd = smalls.tile([128, nq], f32, tag="rstd")
                nc.scalar.activation(rstd, lnv, AF.Exp, bias=0.0, scale=-0.5)
                for i, qt in enumerate(qts):
                    o = lnp.tile([128, H], bf16, tag="o")
                    eng = nc.vector if (alt_engine and qt % 2) else nc.gpsimd
                    eng.tensor_scalar(out=o, in0=ys[qt],
                                      scalar1=mvb[:, qt, 0:1],
                                      scalar2=rstd[:, i:i + 1],
                                      op0=OP.subtract, op1=OP.mult)
                    dge = nc.sync if alt_engine else nc.gpsimd
                    dge.dma_start(out=out_t[b * KT4 + qt], in_=o)

            def emit_scores_exp(b, pr, qb, kb):
                ex = expp.tile([128, KT4, 2 * S], fp8, tag="ex")
                for kt in range(KT4):
                    ps = scp.tile([128, 1024], f32, tag="sc")
                    for hh in range(2):
                        lo, hi = hh * 64, (hh + 1) * 64
                        nc.tensor.matmul(
                            ps[:, hh * 512:(hh + 1) * 512],
                            kb[lo:hi, pr, kt * 128:(kt + 1) * 128],
                            qb[lo:hi, pr, :],
                            start=True, stop=True)
                    nc.scalar.activation(ex[:, kt, :], ps, AF.Exp,
                                         bias=mask_sb[:, kt, b:b + 1],
                                         scale=2.0 ** -5)
                return ex

            def emit_s_chain(ex):
                """Transposed softmax sums sT[q%64, hh*8+qs] = sum_k ex/4 (tiny
                ones-rhs matmuls into the proj psum ring), partition-parallel
                reciprocal, then scatter+broadcast 1/s to a [64, 1024] tile."""
                st = pp.tile([64, 16], f32, tag="proj")
                for hh in range(2):
                    for qs in range(8):
                        c0 = hh * 512 + qs * 64
                        nc.tensor.matmul(
                            st[:, hh * 8 + qs:hh * 8 + qs + 1],
                            ex[:, 0:2, c0:c0 + 64], ones8_sb,
                            start=True, stop=False, perf_mode=PM.DoubleRow)
                        nc.tensor.matmul(
                            st[:, hh * 8 + qs:hh * 8 + qs + 1],
                            ex[:, 2:4, c0:c0 + 64], ones8_sb,
                            start=False, stop=True, perf_mode=PM.DoubleRow)
                str_sb = smalls.tile([64, 16], f32, tag="str", bufs=8)
                nc.vector.reciprocal(str_sb, st)
                # scatter 1/s into final column order dsr[c*64+l], so the
                # broadcast is one clean 2-dim contiguous DMA
                dsr = drp.tile([1, 1024], f32, tag="dsr")
                nc.sync.dma_start(
                    out=bass.AP(tensor=dsr.tensor, offset=dsr.offset,
                                ap=[[1, 64], [64, 16], [1, 1]]),
                    in_=str_sb)
                sbc = sbcp.tile([64, 1024], f32, tag="sbc")
                nc.sync.dma_start(out=sbc, in_=dsr.to_broadcast([64, 1024]))
                return sbc

            def emit_attnv_mult(vb, wt_sb, pr, ex, sbc):
                # both heads into one [64, 1024] psum; single-buffered is fine
                # because attnv(p+1) is emitted a full pair after mult(p)
                wev = wevp.tile([64, 1024], f32, tag="wev", bufs=1)
                for hh in range(2):
                    h = 2 * pr + hh
                    for g in range(2):
                        nc.tensor.matmul(
                            wev[:, hh * 512:(hh + 1) * 512],
                            vb[:, 2 * g:2 * g + 2, h, :],
                            ex[:, 2 * g:2 * g + 2, hh * 512:(hh + 1) * 512],
                            start=(g == 0), stop=(g == 1),
                            perf_mode=PM.DoubleRow)
                nc.vector.tensor_mul(wt_sb[:, 2 * pr:2 * pr + 2, :], wev, sbc)

            # prologue: batch 0 projections
            cur = alloc_qkv()
            for pr in range(PAIRS):
                emit_proj_slice(0, pr, cur)

            pending = None   # o_ln state of the previous batch
            st1 = None       # (vb, wt, pr, ex): awaiting s_chain (1 pair back)
            st2 = None       # (vb, wt, pr, ex, sbc): awaiting attnv (2 back)
            for b in range(B_LOC):
                qb, kb, vb = cur
                nxt = alloc_qkv() if b + 1 < B_LOC else None

                wt_sb = wtp.tile([64, NH, S], fp8, tag="wt")
                xrs = []
                # attn*V + normalize run one pair behind scores/exp (so the
                # 1/s scatter+broadcast DMA chain has a full pair of slack and
                # the DVE stream never head-of-line blocks on it); the
                # PREVIOUS batch's O-projection/LN spreads across pairs 0-4.
                for pr in range(PAIRS):
                    ex = emit_scores_exp(b, pr, qb, kb)
                    if nxt is not None:
                        emit_proj_slice(b + 1, pr, nxt)
                    if st2 is not None:
                        emit_attnv_mult(*st2)
                    if pending is not None:
                        if pr < KT4:
                            emit_o_chunk(pending, pr)
                        elif pr == KT4:
                            emit_fin(pending, range(KT4))
                            pending = None
                    if pr == 3:
                        # batch 0's loads go behind the prologue on sync, and
                        # the 4-deep ring makes later batches' dispatches wait
                        # for consumption — otherwise the Pool DGE issues all
                        # 16 at t=0 and their transfers cut into the DMA FIFO
                        # ahead of the weight loads
                        dge = nc.sync if b == 0 else nc.gpsimd
                        for qt in range(KT4):
                            xr = lnp.tile([128, H], bf16, tag="xr", bufs=4)
                            dge.dma_start(out=xr, in_=xres_t[b * KT4 + qt])
                            xrs.append(xr)
                    st2 = (vb, wt_sb, pr, ex, emit_s_chain(ex))
                    if b == B_LOC - 1:
                        # last batch: flush same-pair (DVE has slack to absorb
                        # the broadcast wait), so the drain starts earlier
                        emit_attnv_mult(*st2)
                        st2 = None

                mvb = smalls.tile([128, KT4, 2], f32, tag="mvb")
                ost = {"b": b, "wt": wt_sb, "xrs": xrs, "ys": [], "mvb": mvb}
                if b < B_LOC - 1:
                    pending = ost
                else:
                    # tail: per-qt finalize on alternating engines/psum rings
                    for qt in range(KT4):
                        emit_o_chunk(ost, qt, split_pp=(qt % 2 == 1))
                        emit_fin(ost, [qt], alt_engine=True)
                cur = nxt

    nc.compile()
    return nc


def _get_nc():
    if "nc" not in _CACHE:
        _CACHE["nc"] = _build()
    return _CACHE["nc"]


def _prep_in_maps(inputs):
    x = np.asarray(inputs["x"], np.float32)
    mask = np.asarray(inputs["additive_attention_mask"], np.float32)
    Wq = np.asarray(inputs["Wq"], np.float32)
    Wk = np.asarray(inputs["Wk"], np.float32)
    Wv = np.asarray(inputs["Wv"], np.float32)
    Wo = np.asarray(inputs["Wo"], np.float32)
    bq = np.asarray(inputs["bq"], np.float32)
    bk = np.asarray(inputs["bk"], np.float32)
    bv = np.asarray(inputs["bv"], np.float32)
    bo = np.asarray(inputs["bo"], np.float32)

    wq8 = np.ascontiguousarray(Wq.T * 64.0).astype(FP8)
    wk8 = np.ascontiguousarray(Wk.T * 64.0).astype(FP8)
    wv8 = np.ascontiguousarray(Wv.T * 64.0).astype(FP8)
    wo8 = np.ascontiguousarray(
        (Wo.T * 64.0).reshape(NH, D, H).transpose(1, 0, 2)).astype(FP8)
    bq2 = (2.0 * bq).reshape(KT, 128).T
    bk2 = (2.0 * bk).reshape(KT, 128).T
    bo2 = bo + Wo @ bv  # attn weights sum to 1: bv passes through to O-proj

    shared = {"wq": wq8, "wk": wk8, "wv": wv8, "wo": wo8}
    in_maps = []
    for c in range(N_CORES):
        xs = x[c * B_LOC:(c + 1) * B_LOC].reshape(T, H)
        # mask as [128, kt, b]: token k = kt*128+p of batch b, minus ln4
        mc = (mask[c * B_LOC:(c + 1) * B_LOC, 0, 0, :] - np.log(4.0))
        mkb = mc.reshape(B_LOC, KT4, 128).transpose(2, 1, 0).reshape(128, KT4 * B_LOC)
        cst = np.concatenate([bq2, bk2, mkb], axis=1).astype(np.float32)
        in_maps.append({
            "xT": np.ascontiguousarray(xs.T).astype(FP8),
            "xres": (np.ascontiguousarray(xs + bo2[None, :]) * 256.0).astype(BF16),
            "cst": np.ascontiguousarray(cst),
            **shared,
        })
    return in_maps


def run(inputs, trace=False):
    """Returns (full_output, BassKernelResults)."""
    from concourse.bass_utils import run_bass_kernel_spmd

    nc = _get_nc()
    in_maps = _prep_in_maps(inputs)
    res = run_bass_kernel_spmd(nc, in_maps, core_ids=list(range(N_CORES)),
                               trace=trace)
    out = np.concatenate(
        [res.results[c]["out"].astype(np.float32).reshape(B_LOC, S, H)
         for c in range(N_CORES)], axis=0)
    ln_w = np.asarray(inputs["ln_w"], np.float32)
    ln_b = np.asarray(inputs["ln_b"], np.float32)
    out = out * ln_w[None, None, :] + ln_b[None, None, :]
    return np.ascontiguousarray(out.astype(np.float32)), res


def kernel(**inputs) -> np.ndarray:
    out, _ = run(inputs, trace=False)
    return out
